# revision 1
# baseline (speedup 1.0000x reference)
"""Multi-head attention Bass/Tile kernel for TRN2, sharded 8 ways.

Sharding: core c handles batch b = c//2 and heads half = c%2 (8 of 16 heads).
Each core computes, for its batch and its 8 heads:
  q/k/v projections -> scoresT = K @ Q^T (per head, [t, s] layout) -> exp ->
  PV matmul with a ones-column appended to V (gives row sums for free) ->
  normalize -> partial output projection against its 512 rows of Wout^T.
Host sums the two partials per batch and adds the bias.

Layout choices (all chosen so NO transposes are needed anywhere):
  xT     [D, S]  : host-pretransposed activations (d on partitions)
  wq/wk  [D, H*dk] : lhsT layout for qT/kT = W^T @ xT
  wv     [D, H*dk] : rhs layout for v = xT^T @ wv  ([t, vdim], natural)
  kT     [H*dk, S]: j on partitions -> head-pair p lives in 128-row chunk p
  qTz    zero-padded per head: scores contract K=128 at base partition 0,
         sharing the kT stationary operand between the pair's two matmuls
  scoresT[t, s]   : lhsT=kT [j,t], rhs=qTz [j,s]; softmax sum over t is
                    folded into the PV matmul via the ones column of v'.
  out    [s, o]   : lhsT=concatT [i,s], rhs=woutT [i,o]

The whole kernel is one software pipeline over units (sb, hp): the PV
matmuls of unit k-1 are interleaved t-chunk-wise into the scores loop of
unit k so the PE never queues behind an exp it is waiting on, and the ACT
engine (the bottleneck: 33.5M exps/core) is fed continuously. The v'
projection fills the PV slot of the very first unit.

HW pitfalls baked in (learned on-device):
  - no partition-shifting DVE copies (sim allows them, HW corrupts);
    the only cross-partition moves are InstReciprocal psum[64:65]->sbuf[0:1]
    (verified on HW) and gpsimd partition_broadcast
  - reciprocal_approx_fast (custom DVE op) produces garbage on HW
  - matmul free dim capped at 512; 2-bank psum tiles need bank-aligned halves
"""

from contextlib import ExitStack
from dataclasses import dataclass

import numpy as np
import ml_dtypes

import concourse.bass as bass  # noqa: F401
import concourse.tile as tile
from concourse import bacc, mybir


@dataclass
class Cfg:
    D: int = 1024      # model dim
    S: int = 2048      # sequence length (queries == keys)
    HL: int = 8        # heads per core
    DK: int = 64       # head dim
    S_BLK: int = 512   # query block (matmul free dim)
    T_BLK: int = 512   # t block in projection phase

    @property
    def DC(self):
        return self.D // 128

    @property
    def NSB(self):
        return self.S // self.S_BLK

    @property
    def TBn(self):
        return self.S // self.T_BLK

    @property
    def TCn(self):
        return self.S // 128

    @property
    def JW(self):
        return self.HL * self.DK

    @property
    def JC(self):
        return self.JW // 128

    @property
    def VW(self):
        return self.DK + 1

    @property
    def OB(self):
        return min(512, self.D)


DT_NP = {
    mybir.dt.bfloat16: ml_dtypes.bfloat16,
    mybir.dt.float32: np.float32,
    mybir.dt.float32r: np.float32,
}


def build_nc(cfg: Cfg, DT=mybir.dt.bfloat16, num_devices: int = 8):
    c = cfg
    f32 = mybir.dt.float32
    EXPDT = DT if DT == mybir.dt.bfloat16 else f32
    SCALE = 1.0 / float(np.sqrt(c.DK))
    nc = bacc.Bacc("TRN2", target_bir_lowering=False, debug=False,
                   num_devices=num_devices)

    xqT = nc.dram_tensor("xqT", [c.D, c.S], DT, kind="ExternalInput").ap()
    xkT = nc.dram_tensor("xkT", [c.D, c.S], DT, kind="ExternalInput").ap()
    xvT = nc.dram_tensor("xvT", [c.D, c.S], DT, kind="ExternalInput").ap()
    wq_d = nc.dram_tensor("wq", [c.D, c.JW], DT, kind="ExternalInput").ap()
    wk_d = nc.dram_tensor("wk", [c.D, c.JW], DT, kind="ExternalInput").ap()
    wv_d = nc.dram_tensor("wv", [c.D, c.JW], DT, kind="ExternalInput").ap()
    wo_d = nc.dram_tensor("woutT", [c.JW, c.D], DT, kind="ExternalInput").ap()
    out_d = nc.dram_tensor("out", [c.S, c.D], f32, kind="ExternalOutput").ap()

    with tile.TileContext(nc) as tc, ExitStack() as es:
        wpool = es.enter_context(tc.tile_pool(name="weights", bufs=1))
        kvpool = es.enter_context(tc.tile_pool(name="kv", bufs=1))
        xpool = es.enter_context(tc.tile_pool(name="x", bufs=2))
        qpool = es.enter_context(tc.tile_pool(name="q", bufs=2))
        epool = es.enter_context(tc.tile_pool(name="exp", bufs=2))
        cpool = es.enter_context(tc.tile_pool(name="cat", bufs=2))
        opool = es.enter_context(tc.tile_pool(name="o", bufs=2))
        rpool = es.enter_context(tc.tile_pool(name="r", bufs=1))
        pspool = es.enter_context(tc.tile_pool(name="ps", bufs=2, space="PSUM"))
        pvpool = es.enter_context(tc.tile_pool(name="pv", bufs=2, space="PSUM"))
        fppool = es.enter_context(tc.tile_pool(name="fp", bufs=2, space="PSUM"))
        stpool = es.enter_context(tc.tile_pool(name="st", bufs=2))

        def load_w_dmaj(dram, width, tag):
            t = wpool.tile([128, c.DC * width], DT, tag=tag, name=tag)
            for d in range(c.DC):
                eng = nc.sync if d % 2 == 0 else nc.gpsimd
                eng.dma_start(t[:, d * width:(d + 1) * width],
                              dram[d * 128:(d + 1) * 128, :])
            return t

        def load_x_blk(dram, blk, width, name):
            t = xpool.tile([128, c.DC * width], DT, tag="x", name=name)
            for d in range(c.DC):
                eng = nc.sync if d % 2 == 0 else nc.gpsimd
                eng.dma_start(
                    t[:, d * width:(d + 1) * width],
                    dram[d * 128:(d + 1) * 128, blk * width:(blk + 1) * width])
            return t

        # ---- kT (wk + first xk DMAs lead the queue) ----
        wk_sb = load_w_dmaj(wk_d, c.JW, "wk")
        kT_sb = kvpool.tile([128, c.JC * c.S], DT)
        NT = c.T_BLK
        for tb in range(c.TBn):
            xk = load_x_blk(xkT, tb, NT, f"xk{tb}")
            for jc in range(c.JC):
                ps = pspool.tile([128, NT], f32, tag="ps", name=f"psk{tb}_{jc}")
                for d in range(c.DC):
                    nc.tensor.matmul(
                        ps[:],
                        wk_sb[:, d * c.JW + jc * 128: d * c.JW + (jc + 1) * 128],
                        xk[:, d * NT:(d + 1) * NT],
                        start=(d == 0), stop=(d == c.DC - 1))
                nc.vector.tensor_copy(
                    kT_sb[:, jc * c.S + tb * NT: jc * c.S + (tb + 1) * NT], ps[:])

        wq_sb = load_w_dmaj(wq_d, c.JW, "wq")

        # v' is emitted later (inside the first pipeline unit); declare here.
        v_sb = kvpool.tile([128, c.TCn * c.HL * c.VW], DT)
        wo_sb_box = {}

        def emit_v_phase():
            wv_sb = load_w_dmaj(wv_d, c.JW, "wv")
            nc.gpsimd.memset(v_sb[:], 1.0)  # ones columns preset
            for tb in range(c.TBn):
                xv = load_x_blk(xvT, tb, NT, f"xv{tb}")
                for tt in range(NT // 128):
                    g = tb * (NT // 128) + tt
                    ps = pspool.tile([128, c.JW], f32, tag="ps", name=f"psv{g}")
                    for d in range(c.DC):
                        nc.tensor.matmul(
                            ps[:],
                            xv[:, d * NT + tt * 128: d * NT + (tt + 1) * 128],
                            wv_sb[:, d * c.JW:(d + 1) * c.JW],
                            start=(d == 0), stop=(d == c.DC - 1))
                    dst = v_sb[:, g * c.HL * c.VW:(g + 1) * c.HL * c.VW]
                    dst3 = dst.rearrange("p (h w) -> p h w", w=c.VW)[:, :, 0:c.DK]
                    src3 = ps[:].rearrange("p (h w) -> p h w", w=c.DK)
                    nc.vector.tensor_copy(dst3, src3)
            # wo load rides behind the v-phase traffic, ahead of first out-proj
            wo_sb = wpool.tile([128, c.JC * c.D], DT, tag="wo", name="wo")
            for ic in range(c.JC):
                nc.sync.dma_start(wo_sb[:, ic * c.D:(ic + 1) * c.D],
                                  wo_d[ic * 128:(ic + 1) * 128, :])
            wo_sb_box["wo"] = wo_sb

        def emit_qT_mms(sb, xq, qTz):
            """Filler closures: 32 MMs; the last per jc writes the zero-padded
            qTz halves (head A rows 0:64 in block 2jc, head B rows 64:128 in
            block 2jc+1, complementary halves zeroed)."""
            ops = []
            psq_box = {}

            def mk(jc, d):
                def op():
                    if d == 0:
                        psq_box[jc] = fppool.tile([128, c.S_BLK], f32, tag="fp",
                                                  name=f"psq{sb}_{jc}")
                    nc.tensor.matmul(
                        psq_box[jc][:],
                        wq_sb[:, d * c.JW + jc * 128: d * c.JW + (jc + 1) * 128],
                        xq[:, d * c.S_BLK:(d + 1) * c.S_BLK],
                        start=(d == 0), stop=(d == c.DC - 1))
                    if d == c.DC - 1:
                        ps = psq_box[jc]
                        ca = (2 * jc) * c.S_BLK
                        cb = (2 * jc + 1) * c.S_BLK
                        nc.vector.tensor_copy(qTz[0:64, ca:ca + c.S_BLK],
                                              ps[0:64, :])
                        nc.gpsimd.memset(qTz[64:128, ca:ca + c.S_BLK], 0.0)
                        nc.vector.tensor_copy(qTz[64:128, cb:cb + c.S_BLK],
                                              ps[64:128, :])
                        nc.gpsimd.memset(qTz[0:64, cb:cb + c.S_BLK], 0.0)
                return op
            for jc in range(c.JC):
                for d in range(c.DC):
                    ops.append(mk(jc, d))
            return ops

        def emit_outproj_mms(sb, catT):
            """Filler closures: per (sc, oc): 4 ic-MMs into a 1-bank psum,
            then copy + DMA."""
            ops = []
            po_box = {}

            def mk(sc, oc, ic):
                def op():
                    if ic == 0:
                        po_box[(sc, oc)] = fppool.tile(
                            [128, c.OB], f32, tag="fp", name=f"po{sb}_{sc}_{oc}")
                    po = po_box[(sc, oc)]
                    nc.tensor.matmul(
                        po[:],
                        catT[:, ic * c.S_BLK + sc * 128:
                             ic * c.S_BLK + (sc + 1) * 128],
                        wo_sb_box["wo"][:, ic * c.D + oc * c.OB:
                                        ic * c.D + (oc + 1) * c.OB],
                        start=(ic == 0), stop=(ic == c.JC - 1))
                    if ic == c.JC - 1:
                        ot = opool.tile([128, c.OB], f32, tag="ot",
                                        name=f"ot{sb}_{sc}_{oc}")
                        nc.vector.tensor_copy(ot[:], po[:])
                        nc.sync.dma_start(
                            out_d[sb * c.S_BLK + sc * 128:
                                  sb * c.S_BLK + (sc + 1) * 128,
                                  oc * c.OB:(oc + 1) * c.OB],
                            ot[:])
                return op
            for sc in range(c.S_BLK // 128):
                for oc in range(c.D // c.OB):
                    for ic in range(c.JC):
                        ops.append(mk(sc, oc, ic))
            return ops

        def emit_pv_chunk(u, t0, nt):
            for t in range(t0, t0 + nt):
                nc.tensor.matmul(
                    u["pvA"][0:c.VW, :],
                    v_sb[:, t * c.HL * c.VW + (2 * u["hp"]) * c.VW:
                         t * c.HL * c.VW + (2 * u["hp"] + 1) * c.VW],
                    u["expA"][:, t * c.S_BLK:(t + 1) * c.S_BLK],
                    start=(t == 0), stop=(t == c.TCn - 1))
            for t in range(t0, t0 + nt):
                nc.tensor.matmul(
                    u["pvB"][0:c.VW, :],
                    v_sb[:, t * c.HL * c.VW + (2 * u["hp"] + 1) * c.VW:
                         t * c.HL * c.VW + (2 * u["hp"] + 2) * c.VW],
                    u["expB"][:, t * c.S_BLK:(t + 1) * c.S_BLK],
                    start=(t == 0), stop=(t == c.TCn - 1))

        def emit_stage(u):
            # copy PV psum -> SBUF staging right away so the psum banks free
            # up for the next unit's PV (normalize then runs off-critical-path)
            sb, hp = u["sb"], u["hp"]
            u["stA"] = stpool.tile([c.VW, c.S_BLK], f32, tag="stA",
                                   name=f"stA{sb}_{hp}")
            u["stB"] = stpool.tile([c.VW, c.S_BLK], f32, tag="stB",
                                   name=f"stB{sb}_{hp}")
            nc.vector.tensor_copy(u["stA"][:], u["pvA"][0:c.VW, :])
            nc.vector.tensor_copy(u["stB"][:], u["pvB"][0:c.VW, :])

        def emit_normalize(u):
            sb, hp = u["sb"], u["hp"]
            stA, stB, catT = u["stA"], u["stB"], u["catT"]
            rtiA = rpool.tile([1, c.S_BLK], f32, tag="rtiA", name=f"rtiA{sb}_{hp}")
            rtiB = rpool.tile([1, c.S_BLK], f32, tag="rtiB", name=f"rtiB{sb}_{hp}")
            # NB: cross-partition (row 64 -> row 0) — verified OK on HW for
            # InstReciprocal specifically.
            nc.vector.reciprocal(rtiA[:], stA[c.DK:c.DK + 1, :])
            nc.vector.reciprocal(rtiB[:], stB[c.DK:c.DK + 1, :])
            rbA = rpool.tile([c.DK, c.S_BLK], f32, tag="rbA", name=f"rbA{sb}_{hp}")
            rbB = rpool.tile([c.DK, c.S_BLK], f32, tag="rbB", name=f"rbB{sb}_{hp}")
            nc.gpsimd.partition_broadcast(rbA[:], rtiA[:])
            nc.gpsimd.partition_broadcast(rbB[:], rtiB[:])
            nc.vector.tensor_mul(
                catT[0:c.DK, hp * c.S_BLK:(hp + 1) * c.S_BLK],
                stA[0:c.DK, :], rbA[:])
            nc.vector.tensor_mul(
                catT[64:64 + c.DK, hp * c.S_BLK:(hp + 1) * c.S_BLK],
                stB[0:c.DK, :], rbB[:])

        # ---- the main (sb, hp) software pipeline with per-th fillers ----
        units = [(sb, hp) for sb in range(c.NSB) for hp in range(c.JC)]
        # fillers[idx] = list of MM closures to interleave into unit idx's
        # scores loop (2 per th).
        fillers = [[] for _ in units]
        prev = None
        qT_tiles = {}
        cat_tiles = {}
        xq_tiles = {0: load_x_blk(xqT, 0, c.S_BLK, "xq0")}

        # sb=0 prologue: qT(0) emitted inline (dedicated MMs)
        qT_tiles[0] = qpool.tile([128, c.JC * 2 * c.S_BLK], DT, tag="qT",
                                 name="qT0")
        for op in emit_qT_mms(0, xq_tiles[0], qT_tiles[0]):
            op()

        for idx, (sb, hp) in enumerate(units):
            if hp == 0:
                cat_tiles[sb] = cpool.tile([128, c.JC * c.S_BLK], DT, tag="cat",
                                           name=f"catT{sb}")
            # xq prefetch two units before the qT fillers consume it
            pf = None
            if sb == 0 and hp == max(0, c.JC - 3):
                pf = 1
            elif sb == 0 and hp == c.JC - 1:
                pf = 2
            elif sb >= 1 and hp == min(2, c.JC - 1):
                pf = sb + 2
            if pf is not None and pf < c.NSB and pf not in xq_tiles:
                xq_tiles[pf] = load_x_blk(xqT, pf, c.S_BLK, f"xq{pf}")
            if sb == 0 and hp == max(0, c.JC - 2) and sb + 1 < c.NSB:
                # sb0: qT(1) fillers in the last two units (after the v phase)
                qT_tiles[1] = qpool.tile([128, c.JC * 2 * c.S_BLK], DT,
                                         tag="qT", name="qT1")
                qops = emit_qT_mms(1, xq_tiles[1], qT_tiles[1])
                fillers[idx] += qops[:16]
                fillers[min(idx + 1, len(units) - 1)] += qops[16:]
            if sb >= 1 and hp == 0 and sb + 1 < c.NSB:
                # steady state: qT(sb+1) fillers in the first two units
                qT_tiles[sb + 1] = qpool.tile([128, c.JC * 2 * c.S_BLK], DT,
                                              tag="qT", name=f"qT{sb + 1}")
                qops = emit_qT_mms(sb + 1, xq_tiles[sb + 1], qT_tiles[sb + 1])
                fillers[idx] += qops[:16]
                fillers[min(idx + 1, len(units) - 1)] += qops[16:]
            catT = cat_tiles[sb]
            qT = qT_tiles[sb]
            cur = {
                "sb": sb, "hp": hp, "catT": catT,
                "expA": epool.tile([128, c.TCn * c.S_BLK], EXPDT, tag="expA",
                                   name=f"expA{sb}_{hp}"),
                "expB": epool.tile([128, c.TCn * c.S_BLK], EXPDT, tag="expB",
                                   name=f"expB{sb}_{hp}"),
            }
            if prev is not None:
                prev["pvA"] = pvpool.tile([128, c.S_BLK], f32, tag="pv",
                                          name=f"pvA{prev['sb']}_{prev['hp']}")
                prev["pvB"] = pvpool.tile([128, c.S_BLK], f32, tag="pv",
                                          name=f"pvB{prev['sb']}_{prev['hp']}")
            flist = fillers[idx]
            fpos = 0
            for th in range(c.TCn // 2):
                psA2 = pspool.tile([128, 2 * c.S_BLK], f32, tag="ps",
                                   name=f"psA2{sb}_{hp}_{th}")
                psB2 = pspool.tile([128, 2 * c.S_BLK], f32, tag="ps",
                                   name=f"psB2{sb}_{hp}_{th}")
                for u in range(2):
                    t = 2 * th + u
                    lhsT = kT_sb[:, hp * c.S + t * 128: hp * c.S + (t + 1) * 128]
                    # zero-padded K=128 pair sharing the kT stationary operand;
                    # keeps the PE dense (HAM-warm) in this ACT-bound phase
                    nc.tensor.matmul(
                        psA2[:, u * c.S_BLK:(u + 1) * c.S_BLK], lhsT,
                        qT[:, (2 * hp) * c.S_BLK:(2 * hp + 1) * c.S_BLK],
                        start=True, stop=True)
                    nc.tensor.matmul(
                        psB2[:, u * c.S_BLK:(u + 1) * c.S_BLK], lhsT,
                        qT[:, (2 * hp + 1) * c.S_BLK:(2 * hp + 2) * c.S_BLK],
                        start=True, stop=True)
                nc.scalar.activation(
                    cur["expA"][:, 2 * th * c.S_BLK:(2 * th + 2) * c.S_BLK],
                    psA2[:], mybir.ActivationFunctionType.Exp, scale=SCALE)
                nc.scalar.activation(
                    cur["expB"][:, 2 * th * c.S_BLK:(2 * th + 2) * c.S_BLK],
                    psB2[:], mybir.ActivationFunctionType.Exp, scale=SCALE)
                if prev is not None:
                    emit_pv_chunk(prev, 2 * th, 2)
                    if th == c.TCn // 2 - 1:
                        # stage immediately: frees the pv psum banks before
                        # the filler copies clog the DVE queue
                        emit_stage(prev)
                # interleave filler MMs evenly across the th loop
                want = (len(flist) * (th + 1)) // (c.TCn // 2)
                while fpos < want:
                    flist[fpos]()
                    fpos += 1
            if prev is not None:
                emit_normalize(prev)
                if prev["hp"] == c.JC - 1:
                    # out-projection of prev's sb becomes fillers of later
                    # units of the current sb (catT complete only now)
                    oops = emit_outproj_mms(prev["sb"], prev["catT"])
                    base = idx + 2 if sb + 1 < c.NSB else idx + 1
                    if base < len(units):
                        fillers[base] += oops[:16]
                        if base + 1 < len(units):
                            fillers[base + 1] += oops[16:]
                        else:
                            fillers[base] += oops[16:]
                    else:
                        for op in oops:
                            op()
            if idx == 0:
                emit_v_phase()
            prev = cur
        # drain the pipeline
        prev["pvA"] = pvpool.tile([128, c.S_BLK], f32, tag="pv", name="pvA_last")
        prev["pvB"] = pvpool.tile([128, c.S_BLK], f32, tag="pv", name="pvB_last")
        emit_pv_chunk(prev, 0, c.TCn)
        emit_stage(prev)
        emit_normalize(prev)
        for op in emit_outproj_mms(prev["sb"], prev["catT"]):
            op()

    nc.compile()
    return nc


def shard_inputs(inputs: dict, cfg: Cfg, DT=mybir.dt.bfloat16):
    """Full inputs -> list of 8 per-core in_maps (numpy)."""
    npdt = DT_NP[DT]
    q, k, v = inputs["queries"], inputs["keys"], inputs["values"]
    Wq, Wk, Wv = inputs["Wq"], inputs["Wk"], inputs["Wv"]
    Wout = inputs["Wout"]
    B = q.shape[0]
    maps = []
    WoutT = np.ascontiguousarray(Wout.T)  # [i, o]
    for core in range(2 * B):
        b, half = divmod(core, 2)
        hs = slice(half * cfg.HL, (half + 1) * cfg.HL)
        i0 = half * cfg.JW
        maps.append({
            "xqT": np.ascontiguousarray(q[b].T).astype(npdt),
            "xkT": np.ascontiguousarray(k[b].T).astype(npdt),
            "xvT": np.ascontiguousarray(v[b].T).astype(npdt),
            "wq": np.ascontiguousarray(
                Wq[hs].transpose(1, 0, 2).reshape(cfg.D, cfg.JW)).astype(npdt),
            "wk": np.ascontiguousarray(
                Wk[hs].transpose(1, 0, 2).reshape(cfg.D, cfg.JW)).astype(npdt),
            "wv": np.ascontiguousarray(
                Wv[hs].transpose(1, 0, 2).reshape(cfg.D, cfg.JW)).astype(npdt),
            "woutT": np.ascontiguousarray(WoutT[i0:i0 + cfg.JW]).astype(npdt),
        })
    return maps


def gather_outputs(results, inputs):
    bout = inputs["bout"]
    B = inputs["queries"].shape[0]
    outs = []
    for b in range(B):
        outs.append(results[2 * b]["out"] + results[2 * b + 1]["out"] + bout)
    return np.stack(outs).astype(np.float32)


def percore_reference(in_map: dict, cfg: Cfg):
    """Numpy reference of what one core should produce (fp32 math)."""
    c = cfg
    xq = in_map["xqT"].astype(np.float32).T   # [S, D]
    xk = in_map["xkT"].astype(np.float32).T
    xv = in_map["xvT"].astype(np.float32).T
    wq = in_map["wq"].astype(np.float32)      # [D, JW]
    wk = in_map["wk"].astype(np.float32)
    wv = in_map["wv"].astype(np.float32)
    wo = in_map["woutT"].astype(np.float32)   # [JW, D]
    q = xq @ wq                               # [S, JW]
    k = xk @ wk
    v = xv @ wv
    cat = np.zeros((c.S, c.JW), dtype=np.float32)
    for h in range(c.HL):
        sl = slice(h * c.DK, (h + 1) * c.DK)
        s = (q[:, sl] @ k[:, sl].T) / np.sqrt(c.DK)
        e = np.exp(s)
        p = e / e.sum(axis=1, keepdims=True)
        cat[:, sl] = p @ v[:, sl]
    return cat @ wo

# ----------------------------------------------------------------------------
# Self-contained entry point: kernel(**inputs) -> full [B, S, D] output.
# ----------------------------------------------------------------------------
_NC_CACHE = {}


def _get_nc():
    key = "attn"
    if key not in _NC_CACHE:
        _NC_CACHE[key] = build_nc(Cfg(), mybir.dt.bfloat16, num_devices=8)
    return _NC_CACHE[key]


def kernel(**inputs):
    """Full (unsharded) inputs -> full [4, 2048, 1024] float32 output.

    Shards across the 8 NeuronCores as (batch x head-half), runs the Bass
    kernel SPMD, and gathers: out[b] = partial(core 2b) + partial(core 2b+1)
    + bias (row-sharded fc_out -> partial-sum reduction at gather time).
    """
    from concourse.bass_utils import run_bass_kernel_spmd

    inputs = {k: np.asarray(v) for k, v in inputs.items()}
    cfg = Cfg()
    nc = _get_nc()
    maps = shard_inputs(inputs, cfg, mybir.dt.bfloat16)
    res = run_bass_kernel_spmd(nc, maps, core_ids=list(range(8)), trace=False)
    return gather_outputs(res.results, inputs)



# revision 4
# speedup vs baseline: 1.0364x; 1.0364x over previous
"""Multi-head attention Bass/Tile kernel for TRN2, sharded 8 ways.

Sharding: core c handles batch b = c//2 and heads half = c%2 (8 of 16 heads).
Each core computes, for its batch and its 8 heads:
  q/k/v projections -> scoresT = K @ Q^T (per head, [t, s] layout) -> exp ->
  PV matmul with a ones-column appended to V (gives row sums for free) ->
  normalize -> partial output projection against its 512 rows of Wout^T.
Host sums the two partials per batch and adds the bias.

Layout choices (all chosen so NO transposes are needed anywhere):
  xT     [D, S]  : host-pretransposed activations (d on partitions)
  wq/wk  [D, H*dk] : lhsT layout for qT/kT = W^T @ xT
  wv     [D, H*dk] : rhs layout for v = xT^T @ wv  ([t, vdim], natural)
  kT     [H*dk, S]: j on partitions -> head-pair p lives in 128-row chunk p
  qTz    zero-padded per head: scores contract K=128 at base partition 0,
         sharing the kT stationary operand between the pair's two matmuls
  scoresT[t, s]   : lhsT=kT [j,t], rhs=qTz [j,s]; softmax sum over t is
                    folded into the PV matmul via the ones column of v'.
  out    [s, o]   : lhsT=concatT [i,s], rhs=woutT [i,o]

The whole kernel is one software pipeline over units (sb, hp): the PV
matmuls of unit k-1 are interleaved t-chunk-wise into the scores loop of
unit k so the PE never queues behind an exp it is waiting on, and the ACT
engine (the bottleneck: 33.5M exps/core) is fed continuously. The v'
projection fills the PV slot of the very first unit.

HW pitfalls baked in (learned on-device):
  - no partition-shifting DVE copies (sim allows them, HW corrupts);
    the only cross-partition moves are InstReciprocal psum[64:65]->sbuf[0:1]
    (verified on HW) and gpsimd partition_broadcast
  - reciprocal_approx_fast (custom DVE op) produces garbage on HW
  - matmul free dim capped at 512; 2-bank psum tiles need bank-aligned halves
"""

from contextlib import ExitStack
from dataclasses import dataclass

import numpy as np
import ml_dtypes

import concourse.bass as bass  # noqa: F401
import concourse.tile as tile
from concourse import bacc, mybir


@dataclass
class Cfg:
    D: int = 1024      # model dim
    S: int = 2048      # sequence length (queries == keys)
    HL: int = 8        # heads per core
    DK: int = 64       # head dim
    S_BLK: int = 512   # query block (matmul free dim)
    T_BLK: int = 512   # t block in projection phase

    @property
    def DC(self):
        return self.D // 128

    @property
    def NSB(self):
        return self.S // self.S_BLK

    @property
    def TBn(self):
        return self.S // self.T_BLK

    @property
    def TCn(self):
        return self.S // 128

    @property
    def JW(self):
        return self.HL * self.DK

    @property
    def JC(self):
        return self.JW // 128

    @property
    def VW(self):
        return self.DK + 1

    @property
    def OB(self):
        return min(512, self.D)


DT_NP = {
    mybir.dt.bfloat16: ml_dtypes.bfloat16,
    mybir.dt.float32: np.float32,
    mybir.dt.float32r: np.float32,
}


def build_nc(cfg: Cfg, DT=mybir.dt.bfloat16, num_devices: int = 8):
    c = cfg
    f32 = mybir.dt.float32
    EXPDT = DT if DT == mybir.dt.bfloat16 else f32
    SCALE = 1.0 / float(np.sqrt(c.DK))
    nc = bacc.Bacc("TRN2", target_bir_lowering=False, debug=False,
                   num_devices=num_devices)

    xqT = nc.dram_tensor("xqT", [c.D, c.S], DT, kind="ExternalInput").ap()
    xkT = nc.dram_tensor("xkT", [c.D, c.S], DT, kind="ExternalInput").ap()
    xvT = nc.dram_tensor("xvT", [c.D, c.S], DT, kind="ExternalInput").ap()
    wq_d = nc.dram_tensor("wq", [c.D, c.JW], DT, kind="ExternalInput").ap()
    wk_d = nc.dram_tensor("wk", [c.D, c.JW], DT, kind="ExternalInput").ap()
    wv_d = nc.dram_tensor("wv", [c.D, c.JW], DT, kind="ExternalInput").ap()
    wo_d = nc.dram_tensor("woutT", [c.JW, c.D], DT, kind="ExternalInput").ap()
    out_d = nc.dram_tensor("out", [c.S, c.D], f32, kind="ExternalOutput").ap()

    with tile.TileContext(nc) as tc, ExitStack() as es:
        wpool = es.enter_context(tc.tile_pool(name="weights", bufs=1))
        kvpool = es.enter_context(tc.tile_pool(name="kv", bufs=1))
        xpool = es.enter_context(tc.tile_pool(name="x", bufs=2))
        qpool = es.enter_context(tc.tile_pool(name="q", bufs=2))
        epool = es.enter_context(tc.tile_pool(name="exp", bufs=2))
        cpool = es.enter_context(tc.tile_pool(name="cat", bufs=2))
        opool = es.enter_context(tc.tile_pool(name="o", bufs=2))
        rpool = es.enter_context(tc.tile_pool(name="r", bufs=1))
        pspool = es.enter_context(tc.tile_pool(name="ps", bufs=2, space="PSUM"))
        pvpool = es.enter_context(tc.tile_pool(name="pv", bufs=2, space="PSUM"))
        fppool = es.enter_context(tc.tile_pool(name="fp", bufs=2, space="PSUM"))
        stpool = es.enter_context(tc.tile_pool(name="st", bufs=2))

        def load_w_dmaj(dram, width, tag):
            t = wpool.tile([128, c.DC * width], DT, tag=tag, name=tag)
            for d in range(c.DC):
                eng = nc.sync if d % 2 == 0 else nc.gpsimd
                eng.dma_start(t[:, d * width:(d + 1) * width],
                              dram[d * 128:(d + 1) * 128, :])
            return t

        def load_x_blk(dram, blk, width, name):
            t = xpool.tile([128, c.DC * width], DT, tag="x", name=name)
            for d in range(c.DC):
                eng = nc.sync if d % 2 == 0 else nc.gpsimd
                eng.dma_start(
                    t[:, d * width:(d + 1) * width],
                    dram[d * 128:(d + 1) * 128, blk * width:(blk + 1) * width])
            return t

        # ---- kT (wk + first xk DMAs lead the queue) ----
        wk_sb = load_w_dmaj(wk_d, c.JW, "wk")
        kT_sb = kvpool.tile([128, c.JC * c.S], DT)
        NT = c.T_BLK
        for tb in range(c.TBn):
            xk = load_x_blk(xkT, tb, NT, f"xk{tb}")
            for jc in range(c.JC):
                ps = pspool.tile([128, NT], f32, tag="ps", name=f"psk{tb}_{jc}")
                for d in range(c.DC):
                    nc.tensor.matmul(
                        ps[:],
                        wk_sb[:, d * c.JW + jc * 128: d * c.JW + (jc + 1) * 128],
                        xk[:, d * NT:(d + 1) * NT],
                        start=(d == 0), stop=(d == c.DC - 1))
                nc.vector.tensor_copy(
                    kT_sb[:, jc * c.S + tb * NT: jc * c.S + (tb + 1) * NT], ps[:])

        wq_sb = load_w_dmaj(wq_d, c.JW, "wq")

        # v' is emitted later (inside the first pipeline unit); declare here.
        v_sb = kvpool.tile([128, c.TCn * c.HL * c.VW], DT)
        wo_sb_box = {}

        def emit_v_phase():
            wv_sb = load_w_dmaj(wv_d, c.JW, "wv")
            nc.gpsimd.memset(v_sb[:], 1.0)  # ones columns preset
            for tb in range(c.TBn):
                xv = load_x_blk(xvT, tb, NT, f"xv{tb}")
                for tt in range(NT // 128):
                    g = tb * (NT // 128) + tt
                    ps = pspool.tile([128, c.JW], f32, tag="ps", name=f"psv{g}")
                    for d in range(c.DC):
                        nc.tensor.matmul(
                            ps[:],
                            xv[:, d * NT + tt * 128: d * NT + (tt + 1) * 128],
                            wv_sb[:, d * c.JW:(d + 1) * c.JW],
                            start=(d == 0), stop=(d == c.DC - 1))
                    dst = v_sb[:, g * c.HL * c.VW:(g + 1) * c.HL * c.VW]
                    dst3 = dst.rearrange("p (h w) -> p h w", w=c.VW)[:, :, 0:c.DK]
                    src3 = ps[:].rearrange("p (h w) -> p h w", w=c.DK)
                    nc.vector.tensor_copy(dst3, src3)
            # wo load rides behind the v-phase traffic, ahead of first out-proj
            wo_sb = wpool.tile([128, c.JC * c.D], DT, tag="wo", name="wo")
            for ic in range(c.JC):
                nc.sync.dma_start(wo_sb[:, ic * c.D:(ic + 1) * c.D],
                                  wo_d[ic * 128:(ic + 1) * 128, :])
            wo_sb_box["wo"] = wo_sb

        def emit_qT_mms(sb, xq, qT):
            """Filler closures: 32 MMs; the last per jc copies psum -> qT
            chunk jc (head A rows 0:64, head B rows 64:128 — natural layout,
            consumed by the row-tiled scores matmuls)."""
            ops = []
            psq_box = {}

            def mk(jc, d):
                def op():
                    if d == 0:
                        psq_box[jc] = fppool.tile([128, c.S_BLK], f32, tag="fp",
                                                  name=f"psq{sb}_{jc}")
                    nc.tensor.matmul(
                        psq_box[jc][:],
                        wq_sb[:, d * c.JW + jc * 128: d * c.JW + (jc + 1) * 128],
                        xq[:, d * c.S_BLK:(d + 1) * c.S_BLK],
                        start=(d == 0), stop=(d == c.DC - 1))
                    if d == c.DC - 1:
                        nc.vector.tensor_copy(
                            qT[:, jc * c.S_BLK:(jc + 1) * c.S_BLK],
                            psq_box[jc][:])
                return op
            for jc in range(c.JC):
                for d in range(c.DC):
                    ops.append(mk(jc, d))
            return ops

        def emit_outproj_mms(sb, catT):
            """Filler closures: per (sc, oc): 4 ic-MMs into a 1-bank psum,
            then copy + DMA."""
            ops = []
            po_box = {}

            def mk(sc, oc, ic):
                def op():
                    if ic == 0:
                        po_box[(sc, oc)] = fppool.tile(
                            [128, c.OB], f32, tag="fp", name=f"po{sb}_{sc}_{oc}")
                    po = po_box[(sc, oc)]
                    nc.tensor.matmul(
                        po[:],
                        catT[:, ic * c.S_BLK + sc * 128:
                             ic * c.S_BLK + (sc + 1) * 128],
                        wo_sb_box["wo"][:, ic * c.D + oc * c.OB:
                                        ic * c.D + (oc + 1) * c.OB],
                        start=(ic == 0), stop=(ic == c.JC - 1))
                    if ic == c.JC - 1:
                        ot = opool.tile([128, c.OB], f32, tag="ot",
                                        name=f"ot{sb}_{sc}_{oc}")
                        nc.vector.tensor_copy(ot[:], po[:])
                        nc.sync.dma_start(
                            out_d[sb * c.S_BLK + sc * 128:
                                  sb * c.S_BLK + (sc + 1) * 128,
                                  oc * c.OB:(oc + 1) * c.OB],
                            ot[:])
                return op
            for sc in range(c.S_BLK // 128):
                for oc in range(c.D // c.OB):
                    for ic in range(c.JC):
                        ops.append(mk(sc, oc, ic))
            return ops

        def emit_pv_chunk(u, t0, nt):
            for t in range(t0, t0 + nt):
                nc.tensor.matmul(
                    u["pvA"][0:c.VW, :],
                    v_sb[:, t * c.HL * c.VW + (2 * u["hp"]) * c.VW:
                         t * c.HL * c.VW + (2 * u["hp"] + 1) * c.VW],
                    u["expA"][:, t * c.S_BLK:(t + 1) * c.S_BLK],
                    start=(t == 0), stop=(t == c.TCn - 1))
            for t in range(t0, t0 + nt):
                nc.tensor.matmul(
                    u["pvB"][0:c.VW, :],
                    v_sb[:, t * c.HL * c.VW + (2 * u["hp"] + 1) * c.VW:
                         t * c.HL * c.VW + (2 * u["hp"] + 2) * c.VW],
                    u["expB"][:, t * c.S_BLK:(t + 1) * c.S_BLK],
                    start=(t == 0), stop=(t == c.TCn - 1))

        def emit_stage(u):
            # copy PV psum -> SBUF staging right away so the psum banks free
            # up for the next unit's PV (normalize then runs off-critical-path)
            sb, hp = u["sb"], u["hp"]
            u["stA"] = stpool.tile([c.VW, c.S_BLK], f32, tag="stA",
                                   name=f"stA{sb}_{hp}")
            u["stB"] = stpool.tile([c.VW, c.S_BLK], f32, tag="stB",
                                   name=f"stB{sb}_{hp}")
            nc.vector.tensor_copy(u["stA"][:], u["pvA"][0:c.VW, :])
            nc.vector.tensor_copy(u["stB"][:], u["pvB"][0:c.VW, :])

        def emit_normalize(u):
            sb, hp = u["sb"], u["hp"]
            stA, stB, catT = u["stA"], u["stB"], u["catT"]
            rtiA = rpool.tile([1, c.S_BLK], f32, tag="rtiA", name=f"rtiA{sb}_{hp}")
            rtiB = rpool.tile([1, c.S_BLK], f32, tag="rtiB", name=f"rtiB{sb}_{hp}")
            # NB: cross-partition (row 64 -> row 0) — verified OK on HW for
            # InstReciprocal specifically.
            nc.vector.reciprocal(rtiA[:], stA[c.DK:c.DK + 1, :])
            nc.vector.reciprocal(rtiB[:], stB[c.DK:c.DK + 1, :])
            rbA = rpool.tile([c.DK, c.S_BLK], f32, tag="rbA", name=f"rbA{sb}_{hp}")
            rbB = rpool.tile([c.DK, c.S_BLK], f32, tag="rbB", name=f"rbB{sb}_{hp}")
            nc.gpsimd.partition_broadcast(rbA[:], rtiA[:])
            nc.gpsimd.partition_broadcast(rbB[:], rtiB[:])
            nc.vector.tensor_mul(
                catT[0:c.DK, hp * c.S_BLK:(hp + 1) * c.S_BLK],
                stA[0:c.DK, :], rbA[:])
            nc.vector.tensor_mul(
                catT[64:64 + c.DK, hp * c.S_BLK:(hp + 1) * c.S_BLK],
                stB[0:c.DK, :], rbB[:])

        # ---- the main (sb, hp) software pipeline with per-th fillers ----
        units = [(sb, hp) for sb in range(c.NSB) for hp in range(c.JC)]
        # fillers[idx] = list of MM closures to interleave into unit idx's
        # scores loop (2 per th).
        fillers = [[] for _ in units]
        prev = None
        qT_tiles = {}
        cat_tiles = {}
        xq_tiles = {0: load_x_blk(xqT, 0, c.S_BLK, "xq0")}

        # sb=0 prologue: qT(0) emitted inline (dedicated MMs)
        qT_tiles[0] = qpool.tile([128, c.JC * c.S_BLK], DT, tag="qT",
                                 name="qT0")
        for op in emit_qT_mms(0, xq_tiles[0], qT_tiles[0]):
            op()

        for idx, (sb, hp) in enumerate(units):
            if hp == 0:
                cat_tiles[sb] = cpool.tile([128, c.JC * c.S_BLK], DT, tag="cat",
                                           name=f"catT{sb}")
            # xq prefetch two units before the qT fillers consume it
            pf = None
            if sb == 0 and hp == max(0, c.JC - 3):
                pf = 1
            elif sb == 0 and hp == c.JC - 1:
                pf = 2
            elif sb >= 1 and hp == min(2, c.JC - 1):
                pf = sb + 2
            if pf is not None and pf < c.NSB and pf not in xq_tiles:
                xq_tiles[pf] = load_x_blk(xqT, pf, c.S_BLK, f"xq{pf}")
            if sb == 0 and hp == max(0, c.JC - 2) and sb + 1 < c.NSB:
                # sb0: qT(1) fillers in the last two units (after the v phase)
                qT_tiles[1] = qpool.tile([128, c.JC * c.S_BLK], DT,
                                         tag="qT", name="qT1")
                qops = emit_qT_mms(1, xq_tiles[1], qT_tiles[1])
                fillers[idx] += qops[:16]
                fillers[min(idx + 1, len(units) - 1)] += qops[16:]
            if sb >= 1 and hp == 0 and sb + 1 < c.NSB:
                # steady state: qT(sb+1) fillers in the first two units
                qT_tiles[sb + 1] = qpool.tile([128, c.JC * c.S_BLK], DT,
                                              tag="qT", name=f"qT{sb + 1}")
                qops = emit_qT_mms(sb + 1, xq_tiles[sb + 1], qT_tiles[sb + 1])
                fillers[idx] += qops[:16]
                fillers[min(idx + 1, len(units) - 1)] += qops[16:]
            catT = cat_tiles[sb]
            qT = qT_tiles[sb]
            cur = {
                "sb": sb, "hp": hp, "catT": catT,
                "expA": epool.tile([128, c.TCn * c.S_BLK], EXPDT, tag="expA",
                                   name=f"expA{sb}_{hp}"),
                "expB": epool.tile([128, c.TCn * c.S_BLK], EXPDT, tag="expB",
                                   name=f"expB{sb}_{hp}"),
            }
            if prev is not None:
                prev["pvA"] = pvpool.tile([128, c.S_BLK], f32, tag="pv",
                                          name=f"pvA{prev['sb']}_{prev['hp']}")
                prev["pvB"] = pvpool.tile([128, c.S_BLK], f32, tag="pv",
                                          name=f"pvB{prev['sb']}_{prev['hp']}")
            flist = fillers[idx]
            fpos = 0
            for th in range(c.TCn // 2):
                psA2 = pspool.tile([128, 2 * c.S_BLK], f32, tag="ps",
                                   name=f"psA2{sb}_{hp}_{th}")
                psB2 = pspool.tile([128, 2 * c.S_BLK], f32, tag="ps",
                                   name=f"psB2{sb}_{hp}_{th}")
                for u in range(2):
                    t = 2 * th + u
                    kcol = slice(hp * c.S + t * 128, hp * c.S + (t + 1) * 128)
                    qcol = slice(hp * c.S_BLK, (hp + 1) * c.S_BLK)
                    # K=64 row-tiled pair: head A on PE rows 0-63 (tile 0,0),
                    # head B on rows 64-127 (tile 64,0) — the two matmuls run
                    # concurrently in separate array row-tiles, and psA2/psB2
                    # sit in different psum banks as row tiling requires.
                    nc.tensor.matmul(
                        psA2[:, u * c.S_BLK:(u + 1) * c.S_BLK],
                        kT_sb[0:64, kcol], qT[0:64, qcol],
                        start=True, stop=True)
                    nc.tensor.matmul(
                        psB2[:, u * c.S_BLK:(u + 1) * c.S_BLK],
                        kT_sb[64:128, kcol], qT[64:128, qcol],
                        start=True, stop=True)
                nc.scalar.activation(
                    cur["expA"][:, 2 * th * c.S_BLK:(2 * th + 2) * c.S_BLK],
                    psA2[:], mybir.ActivationFunctionType.Exp, scale=SCALE)
                nc.scalar.activation(
                    cur["expB"][:, 2 * th * c.S_BLK:(2 * th + 2) * c.S_BLK],
                    psB2[:], mybir.ActivationFunctionType.Exp, scale=SCALE)
                if prev is not None:
                    emit_pv_chunk(prev, 2 * th, 2)
                    if th == c.TCn // 2 - 1:
                        # stage immediately: frees the pv psum banks before
                        # the filler copies clog the DVE queue
                        emit_stage(prev)
                # interleave filler MMs evenly across the th loop
                want = (len(flist) * (th + 1)) // (c.TCn // 2)
                while fpos < want:
                    flist[fpos]()
                    fpos += 1
            if prev is not None:
                emit_normalize(prev)
                if prev["hp"] == c.JC - 1:
                    # out-projection of prev's sb becomes fillers of later
                    # units of the current sb (catT complete only now)
                    oops = emit_outproj_mms(prev["sb"], prev["catT"])
                    base = idx + 2 if sb + 1 < c.NSB else idx + 1
                    if base < len(units):
                        fillers[base] += oops[:16]
                        if base + 1 < len(units):
                            fillers[base + 1] += oops[16:]
                        else:
                            fillers[base] += oops[16:]
                    else:
                        for op in oops:
                            op()
            if idx == 0:
                emit_v_phase()
            prev = cur
        # drain the pipeline
        prev["pvA"] = pvpool.tile([128, c.S_BLK], f32, tag="pv", name="pvA_last")
        prev["pvB"] = pvpool.tile([128, c.S_BLK], f32, tag="pv", name="pvB_last")
        emit_pv_chunk(prev, 0, c.TCn)
        emit_stage(prev)
        emit_normalize(prev)
        for op in emit_outproj_mms(prev["sb"], prev["catT"]):
            op()

    nc.compile()
    return nc


def shard_inputs(inputs: dict, cfg: Cfg, DT=mybir.dt.bfloat16):
    """Full inputs -> list of 8 per-core in_maps (numpy)."""
    npdt = DT_NP[DT]
    q, k, v = inputs["queries"], inputs["keys"], inputs["values"]
    Wq, Wk, Wv = inputs["Wq"], inputs["Wk"], inputs["Wv"]
    Wout = inputs["Wout"]
    B = q.shape[0]
    maps = []
    WoutT = np.ascontiguousarray(Wout.T)  # [i, o]
    for core in range(2 * B):
        b, half = divmod(core, 2)
        hs = slice(half * cfg.HL, (half + 1) * cfg.HL)
        i0 = half * cfg.JW
        maps.append({
            "xqT": np.ascontiguousarray(q[b].T).astype(npdt),
            "xkT": np.ascontiguousarray(k[b].T).astype(npdt),
            "xvT": np.ascontiguousarray(v[b].T).astype(npdt),
            "wq": np.ascontiguousarray(
                Wq[hs].transpose(1, 0, 2).reshape(cfg.D, cfg.JW)).astype(npdt),
            "wk": np.ascontiguousarray(
                Wk[hs].transpose(1, 0, 2).reshape(cfg.D, cfg.JW)).astype(npdt),
            "wv": np.ascontiguousarray(
                Wv[hs].transpose(1, 0, 2).reshape(cfg.D, cfg.JW)).astype(npdt),
            "woutT": np.ascontiguousarray(WoutT[i0:i0 + cfg.JW]).astype(npdt),
        })
    return maps


def gather_outputs(results, inputs):
    bout = inputs["bout"]
    B = inputs["queries"].shape[0]
    outs = []
    for b in range(B):
        outs.append(results[2 * b]["out"] + results[2 * b + 1]["out"] + bout)
    return np.stack(outs).astype(np.float32)


def percore_reference(in_map: dict, cfg: Cfg):
    """Numpy reference of what one core should produce (fp32 math)."""
    c = cfg
    xq = in_map["xqT"].astype(np.float32).T   # [S, D]
    xk = in_map["xkT"].astype(np.float32).T
    xv = in_map["xvT"].astype(np.float32).T
    wq = in_map["wq"].astype(np.float32)      # [D, JW]
    wk = in_map["wk"].astype(np.float32)
    wv = in_map["wv"].astype(np.float32)
    wo = in_map["woutT"].astype(np.float32)   # [JW, D]
    q = xq @ wq                               # [S, JW]
    k = xk @ wk
    v = xv @ wv
    cat = np.zeros((c.S, c.JW), dtype=np.float32)
    for h in range(c.HL):
        sl = slice(h * c.DK, (h + 1) * c.DK)
        s = (q[:, sl] @ k[:, sl].T) / np.sqrt(c.DK)
        e = np.exp(s)
        p = e / e.sum(axis=1, keepdims=True)
        cat[:, sl] = p @ v[:, sl]
    return cat @ wo

# ----------------------------------------------------------------------------
# Self-contained entry point: kernel(**inputs) -> full [B, S, D] output.
# ----------------------------------------------------------------------------
_NC_CACHE = {}


def _get_nc():
    key = "attn"
    if key not in _NC_CACHE:
        _NC_CACHE[key] = build_nc(Cfg(), mybir.dt.bfloat16, num_devices=8)
    return _NC_CACHE[key]


def kernel(**inputs):
    """Full (unsharded) inputs -> full [4, 2048, 1024] float32 output.

    Shards across the 8 NeuronCores as (batch x head-half), runs the Bass
    kernel SPMD, and gathers: out[b] = partial(core 2b) + partial(core 2b+1)
    + bias (row-sharded fc_out -> partial-sum reduction at gather time).
    """
    from concourse.bass_utils import run_bass_kernel_spmd

    inputs = {k: np.asarray(v) for k, v in inputs.items()}
    cfg = Cfg()
    nc = _get_nc()
    maps = shard_inputs(inputs, cfg, mybir.dt.bfloat16)
    res = run_bass_kernel_spmd(nc, maps, core_ids=list(range(8)), trace=False)
    return gather_outputs(res.results, inputs)



# revision 7
# speedup vs baseline: 1.1801x; 1.1387x over previous
"""Multi-head attention Bass/Tile kernel for TRN2, sharded 8 ways.

Sharding: core c handles batch b = c//2 and heads half = c%2 (8 of 16 heads).
Each core computes, for its batch and its 8 heads:
  q/k/v projections -> scoresT = K @ Q^T (per head, [t, s] layout) -> exp ->
  PV matmul with a ones-column appended to V (gives row sums for free) ->
  normalize -> partial output projection against its 512 rows of Wout^T.
Host sums the two partials per batch and adds the bias.

Layout choices (all chosen so NO transposes are needed anywhere):
  xT     [D, S]  : host-pretransposed activations (d on partitions)
  wq/wk  [D, H*dk] : lhsT layout for qT/kT = W^T @ xT
  wv     [D, H*dk] : rhs layout for v = xT^T @ wv  ([t, vdim], natural)
  kT     [H*dk, S]: j on partitions -> head-pair p lives in 128-row chunk p
  qTz    zero-padded per head: scores contract K=128 at base partition 0,
         sharing the kT stationary operand between the pair's two matmuls
  scoresT[t, s]   : lhsT=kT [j,t], rhs=qTz [j,s]; softmax sum over t is
                    folded into the PV matmul via the ones column of v'.
  out    [s, o]   : lhsT=concatT [i,s], rhs=woutT [i,o]

The whole kernel is one software pipeline over units (sb, hp): the PV
matmuls of unit k-1 are interleaved t-chunk-wise into the scores loop of
unit k so the PE never queues behind an exp it is waiting on, and the ACT
engine (the bottleneck: 33.5M exps/core) is fed continuously. The v'
projection fills the PV slot of the very first unit.

HW pitfalls baked in (learned on-device):
  - no partition-shifting DVE copies (sim allows them, HW corrupts);
    the only cross-partition moves are InstReciprocal psum[64:65]->sbuf[0:1]
    (verified on HW) and gpsimd partition_broadcast
  - reciprocal_approx_fast (custom DVE op) produces garbage on HW
  - matmul free dim capped at 512; 2-bank psum tiles need bank-aligned halves
"""

from contextlib import ExitStack
from dataclasses import dataclass

import numpy as np
import ml_dtypes

import concourse.bass as bass  # noqa: F401
import concourse.tile as tile
from concourse import bacc, mybir


@dataclass
class Cfg:
    D: int = 1024      # model dim
    S: int = 2048      # sequence length (queries == keys)
    HL: int = 8        # heads per core
    DK: int = 64       # head dim
    S_BLK: int = 512   # query block (matmul free dim)
    T_BLK: int = 512   # t block in projection phase

    @property
    def DC(self):
        return self.D // 128

    @property
    def NSB(self):
        return self.S // self.S_BLK

    @property
    def TBn(self):
        return self.S // self.T_BLK

    @property
    def TCn(self):
        return self.S // 128

    @property
    def JW(self):
        return self.HL * self.DK

    @property
    def JC(self):
        return self.JW // 128

    @property
    def VW(self):
        return self.DK + 1

    @property
    def OB(self):
        return min(512, self.D)


DT_NP = {
    mybir.dt.bfloat16: ml_dtypes.bfloat16,
    mybir.dt.float32: np.float32,
    mybir.dt.float32r: np.float32,
}


def build_nc(cfg: Cfg, DT=mybir.dt.bfloat16, num_devices: int = 8):
    c = cfg
    f32 = mybir.dt.float32
    EXPDT = DT if DT == mybir.dt.bfloat16 else f32
    SCALE = 1.0 / float(np.sqrt(c.DK))
    nc = bacc.Bacc("TRN2", target_bir_lowering=False, debug=False,
                   num_devices=num_devices)

    xqT = nc.dram_tensor("xqT", [c.D, c.S], DT, kind="ExternalInput").ap()
    xkT = nc.dram_tensor("xkT", [c.D, c.S], DT, kind="ExternalInput").ap()
    xvT = nc.dram_tensor("xvT", [c.D, c.S], DT, kind="ExternalInput").ap()
    wq_d = nc.dram_tensor("wq", [c.D, c.JW], DT, kind="ExternalInput").ap()
    wk_d = nc.dram_tensor("wk", [c.D, c.JW], DT, kind="ExternalInput").ap()
    wv_d = nc.dram_tensor("wv", [c.D, c.JW], DT, kind="ExternalInput").ap()
    wo_d = nc.dram_tensor("woutT", [c.JW, c.D], DT, kind="ExternalInput").ap()
    out_d = nc.dram_tensor("out", [c.S, c.D], f32, kind="ExternalOutput").ap()

    with tile.TileContext(nc) as tc, ExitStack() as es:
        wpool = es.enter_context(tc.tile_pool(name="weights", bufs=1))
        kvpool = es.enter_context(tc.tile_pool(name="kv", bufs=1))
        xpool = es.enter_context(tc.tile_pool(name="x", bufs=2))
        qpool = es.enter_context(tc.tile_pool(name="q", bufs=2))
        epool = es.enter_context(tc.tile_pool(name="exp", bufs=2))
        cpool = es.enter_context(tc.tile_pool(name="cat", bufs=2))
        opool = es.enter_context(tc.tile_pool(name="o", bufs=2))
        rpool = es.enter_context(tc.tile_pool(name="r", bufs=1))
        pspool = es.enter_context(tc.tile_pool(name="ps", bufs=2, space="PSUM"))
        pvpool = es.enter_context(tc.tile_pool(name="pv", bufs=2, space="PSUM"))
        fppool = es.enter_context(tc.tile_pool(name="fp", bufs=2, space="PSUM"))
        stpool = es.enter_context(tc.tile_pool(name="st", bufs=2))

        def load_w_dmaj(dram, width, tag):
            t = wpool.tile([128, c.DC * width], DT, tag=tag, name=tag)
            for d in range(c.DC):
                eng = nc.sync if d % 2 == 0 else nc.gpsimd
                eng.dma_start(t[:, d * width:(d + 1) * width],
                              dram[d * 128:(d + 1) * 128, :])
            return t

        def load_x_blk(dram, blk, width, name):
            t = xpool.tile([128, c.DC * width], DT, tag="x", name=name)
            for d in range(c.DC):
                eng = nc.sync if d % 2 == 0 else nc.gpsimd
                eng.dma_start(
                    t[:, d * width:(d + 1) * width],
                    dram[d * 128:(d + 1) * 128, blk * width:(blk + 1) * width])
            return t

        # ---- kT (wk + first xk DMAs lead the queue) ----
        wk_sb = load_w_dmaj(wk_d, c.JW, "wk")
        kT_sb = kvpool.tile([128, c.JC * c.S], DT)
        NT = c.T_BLK
        for tb in range(c.TBn):
            xk = load_x_blk(xkT, tb, NT, f"xk{tb}")
            for jc in range(c.JC):
                ps = pspool.tile([128, NT], f32, tag="ps", name=f"psk{tb}_{jc}")
                for d in range(c.DC):
                    nc.tensor.matmul(
                        ps[:],
                        wk_sb[:, d * c.JW + jc * 128: d * c.JW + (jc + 1) * 128],
                        xk[:, d * NT:(d + 1) * NT],
                        start=(d == 0), stop=(d == c.DC - 1))
                nc.vector.tensor_copy(
                    kT_sb[:, jc * c.S + tb * NT: jc * c.S + (tb + 1) * NT], ps[:])

        wq_sb = load_w_dmaj(wq_d, c.JW, "wq")

        # v' is emitted later (inside the first pipeline unit); declare here.
        v_sb = kvpool.tile([128, c.TCn * c.HL * c.VW], DT)
        wo_sb_box = {}

        def emit_v_phase():
            wv_sb = load_w_dmaj(wv_d, c.JW, "wv")
            nc.gpsimd.memset(v_sb[:], 1.0)  # ones columns preset
            for tb in range(c.TBn):
                xv = load_x_blk(xvT, tb, NT, f"xv{tb}")
                for tt in range(NT // 128):
                    g = tb * (NT // 128) + tt
                    ps = pspool.tile([128, c.JW], f32, tag="ps", name=f"psv{g}")
                    for d in range(c.DC):
                        nc.tensor.matmul(
                            ps[:],
                            xv[:, d * NT + tt * 128: d * NT + (tt + 1) * 128],
                            wv_sb[:, d * c.JW:(d + 1) * c.JW],
                            start=(d == 0), stop=(d == c.DC - 1))
                    dst = v_sb[:, g * c.HL * c.VW:(g + 1) * c.HL * c.VW]
                    dst3 = dst.rearrange("p (h w) -> p h w", w=c.VW)[:, :, 0:c.DK]
                    src3 = ps[:].rearrange("p (h w) -> p h w", w=c.DK)
                    nc.vector.tensor_copy(dst3, src3)
            # wo load rides behind the v-phase traffic, ahead of first out-proj
            wo_sb = wpool.tile([128, c.JC * c.D], DT, tag="wo", name="wo")
            for ic in range(c.JC):
                nc.sync.dma_start(wo_sb[:, ic * c.D:(ic + 1) * c.D],
                                  wo_d[ic * 128:(ic + 1) * 128, :])
            wo_sb_box["wo"] = wo_sb

        def emit_qT_mms(sb, xq, qT):
            """Filler closures: 32 MMs; the last per jc copies psum -> qT
            chunk jc (head A rows 0:64, head B rows 64:128 — natural layout,
            consumed by the row-tiled scores matmuls)."""
            ops = []
            psq_box = {}

            def mk(jc, d):
                def op():
                    if d == 0:
                        psq_box[jc] = fppool.tile([128, c.S_BLK], f32, tag="fp",
                                                  name=f"psq{sb}_{jc}")
                    nc.tensor.matmul(
                        psq_box[jc][:],
                        wq_sb[:, d * c.JW + jc * 128: d * c.JW + (jc + 1) * 128],
                        xq[:, d * c.S_BLK:(d + 1) * c.S_BLK],
                        start=(d == 0), stop=(d == c.DC - 1))
                    if d == c.DC - 1:
                        nc.vector.tensor_copy(
                            qT[:, jc * c.S_BLK:(jc + 1) * c.S_BLK],
                            psq_box[jc][:])
                return op
            for jc in range(c.JC):
                for d in range(c.DC):
                    ops.append(mk(jc, d))
            return ops

        def emit_outproj_mms(sb, catT):
            """Filler closures: per (sc, oc): 4 ic-MMs into a 1-bank psum,
            then copy + DMA."""
            ops = []
            po_box = {}

            def mk(sc, oc, ic):
                def op():
                    if ic == 0:
                        po_box[(sc, oc)] = fppool.tile(
                            [128, c.OB], f32, tag="fp", name=f"po{sb}_{sc}_{oc}")
                    po = po_box[(sc, oc)]
                    nc.tensor.matmul(
                        po[:],
                        catT[:, ic * c.S_BLK + sc * 128:
                             ic * c.S_BLK + (sc + 1) * 128],
                        wo_sb_box["wo"][:, ic * c.D + oc * c.OB:
                                        ic * c.D + (oc + 1) * c.OB],
                        start=(ic == 0), stop=(ic == c.JC - 1))
                    if ic == c.JC - 1:
                        ot = opool.tile([128, c.OB], f32, tag="ot",
                                        name=f"ot{sb}_{sc}_{oc}")
                        nc.vector.tensor_copy(ot[:], po[:])
                        nc.sync.dma_start(
                            out_d[sb * c.S_BLK + sc * 128:
                                  sb * c.S_BLK + (sc + 1) * 128,
                                  oc * c.OB:(oc + 1) * c.OB],
                            ot[:])
                return op
            for sc in range(c.S_BLK // 128):
                for oc in range(c.D // c.OB):
                    for ic in range(c.JC):
                        ops.append(mk(sc, oc, ic))
            return ops

        def emit_pv_chunk(u, t0, nt):
            W2 = 2 * c.S_BLK
            for t in range(t0, t0 + nt):
                nc.tensor.matmul(
                    u["pvA"][0:c.VW, :],
                    v_sb[:, t * c.HL * c.VW + (2 * u["hp"]) * c.VW:
                         t * c.HL * c.VW + (2 * u["hp"] + 1) * c.VW],
                    u["expAB"][:, t * W2:t * W2 + c.S_BLK],
                    start=(t == 0), stop=(t == c.TCn - 1))
            for t in range(t0, t0 + nt):
                nc.tensor.matmul(
                    u["pvB"][0:c.VW, :],
                    v_sb[:, t * c.HL * c.VW + (2 * u["hp"] + 1) * c.VW:
                         t * c.HL * c.VW + (2 * u["hp"] + 2) * c.VW],
                    u["expAB"][:, t * W2 + c.S_BLK:(t + 1) * W2],
                    start=(t == 0), stop=(t == c.TCn - 1))

        def emit_stage(u):
            # copy PV psum -> SBUF staging right away so the psum banks free
            # up for the next unit's PV (normalize then runs off-critical-path)
            sb, hp = u["sb"], u["hp"]
            u["stA"] = stpool.tile([c.VW, c.S_BLK], f32, tag="stA",
                                   name=f"stA{sb}_{hp}")
            u["stB"] = stpool.tile([c.VW, c.S_BLK], f32, tag="stB",
                                   name=f"stB{sb}_{hp}")
            nc.vector.tensor_copy(u["stA"][:], u["pvA"][0:c.VW, :])
            nc.vector.tensor_copy(u["stB"][:], u["pvB"][0:c.VW, :])

        def emit_normalize(u):
            sb, hp = u["sb"], u["hp"]
            stA, stB, catT = u["stA"], u["stB"], u["catT"]
            rtiA = rpool.tile([1, c.S_BLK], f32, tag="rtiA", name=f"rtiA{sb}_{hp}")
            rtiB = rpool.tile([1, c.S_BLK], f32, tag="rtiB", name=f"rtiB{sb}_{hp}")
            # NB: cross-partition (row 64 -> row 0) — verified OK on HW for
            # InstReciprocal specifically.
            nc.vector.reciprocal(rtiA[:], stA[c.DK:c.DK + 1, :])
            nc.vector.reciprocal(rtiB[:], stB[c.DK:c.DK + 1, :])
            rbA = rpool.tile([c.DK, c.S_BLK], f32, tag="rbA", name=f"rbA{sb}_{hp}")
            rbB = rpool.tile([c.DK, c.S_BLK], f32, tag="rbB", name=f"rbB{sb}_{hp}")
            nc.gpsimd.partition_broadcast(rbA[:], rtiA[:])
            nc.gpsimd.partition_broadcast(rbB[:], rtiB[:])
            nc.vector.tensor_mul(
                catT[0:c.DK, hp * c.S_BLK:(hp + 1) * c.S_BLK],
                stA[0:c.DK, :], rbA[:])
            nc.vector.tensor_mul(
                catT[64:64 + c.DK, hp * c.S_BLK:(hp + 1) * c.S_BLK],
                stB[0:c.DK, :], rbB[:])

        # ---- the main (sb, hp) software pipeline with per-th fillers ----
        units = [(sb, hp) for sb in range(c.NSB) for hp in range(c.JC)]
        # fillers[idx] = list of MM closures to interleave into unit idx's
        # scores loop (2 per th).
        fillers = [[] for _ in units]
        prev = None
        qT_tiles = {}
        cat_tiles = {}
        xq_tiles = {0: load_x_blk(xqT, 0, c.S_BLK, "xq0")}

        # sb=0 prologue: qT(0) emitted inline (dedicated MMs)
        qT_tiles[0] = qpool.tile([128, c.JC * c.S_BLK], DT, tag="qT",
                                 name="qT0")
        for op in emit_qT_mms(0, xq_tiles[0], qT_tiles[0]):
            op()

        for idx, (sb, hp) in enumerate(units):
            if hp == 0:
                cat_tiles[sb] = cpool.tile([128, c.JC * c.S_BLK], DT, tag="cat",
                                           name=f"catT{sb}")
            # xq prefetch two units before the qT fillers consume it
            pf = None
            if sb == 0 and hp == max(0, c.JC - 3):
                pf = 1
            elif sb == 0 and hp == c.JC - 1:
                pf = 2
            elif sb >= 1 and hp == min(2, c.JC - 1):
                pf = sb + 2
            if pf is not None and pf < c.NSB and pf not in xq_tiles:
                xq_tiles[pf] = load_x_blk(xqT, pf, c.S_BLK, f"xq{pf}")
            if sb == 0 and hp == max(0, c.JC - 2) and sb + 1 < c.NSB:
                # sb0: qT(1) fillers in the last two units (after the v phase)
                qT_tiles[1] = qpool.tile([128, c.JC * c.S_BLK], DT,
                                         tag="qT", name="qT1")
                qops = emit_qT_mms(1, xq_tiles[1], qT_tiles[1])
                fillers[idx] += qops[:16]
                fillers[min(idx + 1, len(units) - 1)] += qops[16:]
            if sb >= 1 and hp == 0 and sb + 1 < c.NSB:
                # steady state: qT(sb+1) fillers in the first two units
                qT_tiles[sb + 1] = qpool.tile([128, c.JC * c.S_BLK], DT,
                                              tag="qT", name=f"qT{sb + 1}")
                qops = emit_qT_mms(sb + 1, xq_tiles[sb + 1], qT_tiles[sb + 1])
                fillers[idx] += qops[:16]
                fillers[min(idx + 1, len(units) - 1)] += qops[16:]
            catT = cat_tiles[sb]
            qT = qT_tiles[sb]
            cur = {
                "sb": sb, "hp": hp, "catT": catT,
                # per-chunk interleaved [expA(512) | expB(512)] blocks; one
                # activation per chunk covers both heads so a single sem frees
                # the A+B score matmuls of a later chunk together (keeps the
                # row-tiled pair adjacent in the schedule -> concurrent tiles)
                "expAB": epool.tile([128, c.TCn * 2 * c.S_BLK], EXPDT,
                                    tag="expAB", name=f"expAB{sb}_{hp}"),
            }
            if prev is not None:
                prev["pvA"] = pvpool.tile([128, c.S_BLK], f32, tag="pv",
                                          name=f"pvA{prev['sb']}_{prev['hp']}")
                prev["pvB"] = pvpool.tile([128, c.S_BLK], f32, tag="pv",
                                          name=f"pvB{prev['sb']}_{prev['hp']}")
            flist = fillers[idx]
            fpos = 0
            for th in range(c.TCn // 2):
                for u in range(2):
                    t = 2 * th + u
                    kcol = slice(hp * c.S + t * 128, hp * c.S + (t + 1) * 128)
                    qcol = slice(hp * c.S_BLK, (hp + 1) * c.S_BLK)
                    # One 2-bank psum tile [A(512) | B(512)] per t-chunk:
                    # K=64 row-tiled pair (head A on PE rows 0-63, tile (0,0);
                    # head B on rows 64-127, tile (64,0)) lands in different
                    # banks of the same tile, and ONE exp covers both heads.
                    ps2 = pspool.tile([128, 2 * c.S_BLK], f32, tag="ps",
                                      name=f"ps2_{sb}_{hp}_{t}")
                    nc.tensor.matmul(
                        ps2[:, 0:c.S_BLK],
                        kT_sb[0:64, kcol], qT[0:64, qcol],
                        start=True, stop=True)
                    nc.tensor.matmul(
                        ps2[:, c.S_BLK:2 * c.S_BLK],
                        kT_sb[64:128, kcol], qT[64:128, qcol],
                        start=True, stop=True)
                    nc.scalar.activation(
                        cur["expAB"][:, t * 2 * c.S_BLK:(t + 1) * 2 * c.S_BLK],
                        ps2[:], mybir.ActivationFunctionType.Exp, scale=SCALE)
                if prev is not None:
                    emit_pv_chunk(prev, 2 * th, 2)
                    if th == c.TCn // 2 - 1:
                        # stage immediately: frees the pv psum banks before
                        # the filler copies clog the DVE queue
                        emit_stage(prev)
                # interleave filler MMs evenly across the th loop
                want = (len(flist) * (th + 1)) // (c.TCn // 2)
                while fpos < want:
                    flist[fpos]()
                    fpos += 1
            if prev is not None:
                emit_normalize(prev)
                if prev["hp"] == c.JC - 1:
                    # out-projection of prev's sb becomes fillers of later
                    # units of the current sb (catT complete only now)
                    oops = emit_outproj_mms(prev["sb"], prev["catT"])
                    base = idx + 2 if sb + 1 < c.NSB else idx + 1
                    if base < len(units):
                        fillers[base] += oops[:16]
                        if base + 1 < len(units):
                            fillers[base + 1] += oops[16:]
                        else:
                            fillers[base] += oops[16:]
                    else:
                        for op in oops:
                            op()
            if idx == 0:
                emit_v_phase()
            prev = cur
        # drain the pipeline
        prev["pvA"] = pvpool.tile([128, c.S_BLK], f32, tag="pv", name="pvA_last")
        prev["pvB"] = pvpool.tile([128, c.S_BLK], f32, tag="pv", name="pvB_last")
        emit_pv_chunk(prev, 0, c.TCn)
        emit_stage(prev)
        emit_normalize(prev)
        for op in emit_outproj_mms(prev["sb"], prev["catT"]):
            op()

    nc.compile()
    return nc


def shard_inputs(inputs: dict, cfg: Cfg, DT=mybir.dt.bfloat16):
    """Full inputs -> list of 8 per-core in_maps (numpy)."""
    npdt = DT_NP[DT]
    q, k, v = inputs["queries"], inputs["keys"], inputs["values"]
    Wq, Wk, Wv = inputs["Wq"], inputs["Wk"], inputs["Wv"]
    Wout = inputs["Wout"]
    B = q.shape[0]
    maps = []
    WoutT = np.ascontiguousarray(Wout.T)  # [i, o]
    for core in range(2 * B):
        b, half = divmod(core, 2)
        hs = slice(half * cfg.HL, (half + 1) * cfg.HL)
        i0 = half * cfg.JW
        maps.append({
            "xqT": np.ascontiguousarray(q[b].T).astype(npdt),
            "xkT": np.ascontiguousarray(k[b].T).astype(npdt),
            "xvT": np.ascontiguousarray(v[b].T).astype(npdt),
            "wq": np.ascontiguousarray(
                Wq[hs].transpose(1, 0, 2).reshape(cfg.D, cfg.JW)).astype(npdt),
            "wk": np.ascontiguousarray(
                Wk[hs].transpose(1, 0, 2).reshape(cfg.D, cfg.JW)).astype(npdt),
            "wv": np.ascontiguousarray(
                Wv[hs].transpose(1, 0, 2).reshape(cfg.D, cfg.JW)).astype(npdt),
            "woutT": np.ascontiguousarray(WoutT[i0:i0 + cfg.JW]).astype(npdt),
        })
    return maps


def gather_outputs(results, inputs):
    bout = inputs["bout"]
    B = inputs["queries"].shape[0]
    outs = []
    for b in range(B):
        outs.append(results[2 * b]["out"] + results[2 * b + 1]["out"] + bout)
    return np.stack(outs).astype(np.float32)


def percore_reference(in_map: dict, cfg: Cfg):
    """Numpy reference of what one core should produce (fp32 math)."""
    c = cfg
    xq = in_map["xqT"].astype(np.float32).T   # [S, D]
    xk = in_map["xkT"].astype(np.float32).T
    xv = in_map["xvT"].astype(np.float32).T
    wq = in_map["wq"].astype(np.float32)      # [D, JW]
    wk = in_map["wk"].astype(np.float32)
    wv = in_map["wv"].astype(np.float32)
    wo = in_map["woutT"].astype(np.float32)   # [JW, D]
    q = xq @ wq                               # [S, JW]
    k = xk @ wk
    v = xv @ wv
    cat = np.zeros((c.S, c.JW), dtype=np.float32)
    for h in range(c.HL):
        sl = slice(h * c.DK, (h + 1) * c.DK)
        s = (q[:, sl] @ k[:, sl].T) / np.sqrt(c.DK)
        e = np.exp(s)
        p = e / e.sum(axis=1, keepdims=True)
        cat[:, sl] = p @ v[:, sl]
    return cat @ wo

# ----------------------------------------------------------------------------
# Self-contained entry point: kernel(**inputs) -> full [B, S, D] output.
# ----------------------------------------------------------------------------
_NC_CACHE = {}


def _get_nc():
    key = "attn"
    if key not in _NC_CACHE:
        _NC_CACHE[key] = build_nc(Cfg(), mybir.dt.bfloat16, num_devices=8)
    return _NC_CACHE[key]


def kernel(**inputs):
    """Full (unsharded) inputs -> full [4, 2048, 1024] float32 output.

    Shards across the 8 NeuronCores as (batch x head-half), runs the Bass
    kernel SPMD, and gathers: out[b] = partial(core 2b) + partial(core 2b+1)
    + bias (row-sharded fc_out -> partial-sum reduction at gather time).
    """
    from concourse.bass_utils import run_bass_kernel_spmd

    inputs = {k: np.asarray(v) for k, v in inputs.items()}
    cfg = Cfg()
    nc = _get_nc()
    maps = shard_inputs(inputs, cfg, mybir.dt.bfloat16)
    res = run_bass_kernel_spmd(nc, maps, core_ids=list(range(8)), trace=False)
    return gather_outputs(res.results, inputs)



# revision 9
# speedup vs baseline: 1.1939x; 1.0117x over previous
"""Multi-head attention Bass/Tile kernel for TRN2, sharded 8 ways.

Sharding: core c handles batch b = c//2 and heads half = c%2 (8 of 16 heads).
Each core computes, for its batch and its 8 heads:
  q/k/v projections -> scoresT = K @ Q^T (per head, [t, s] layout) -> exp ->
  PV matmul with a ones-column appended to V (gives row sums for free) ->
  normalize -> partial output projection against its 512 rows of Wout^T.
Host sums the two partials per batch and adds the bias.

Layout choices (all chosen so NO transposes are needed anywhere):
  xT     [D, S]  : host-pretransposed activations (d on partitions)
  wq/wk  [D, H*dk] : lhsT layout for qT/kT = W^T @ xT
  wv     [D, H*dk] : rhs layout for v = xT^T @ wv  ([t, vdim], natural)
  kT     [H*dk, S]: j on partitions -> head-pair p lives in 128-row chunk p
  qTz    zero-padded per head: scores contract K=128 at base partition 0,
         sharing the kT stationary operand between the pair's two matmuls
  scoresT[t, s]   : lhsT=kT [j,t], rhs=qTz [j,s]; softmax sum over t is
                    folded into the PV matmul via the ones column of v'.
  out    [s, o]   : lhsT=concatT [i,s], rhs=woutT [i,o]

The whole kernel is one software pipeline over units (sb, hp): the PV
matmuls of unit k-1 are interleaved t-chunk-wise into the scores loop of
unit k so the PE never queues behind an exp it is waiting on, and the ACT
engine (the bottleneck: 33.5M exps/core) is fed continuously. The v'
projection fills the PV slot of the very first unit.

HW pitfalls baked in (learned on-device):
  - no partition-shifting DVE copies (sim allows them, HW corrupts);
    the only cross-partition moves are InstReciprocal psum[64:65]->sbuf[0:1]
    (verified on HW) and gpsimd partition_broadcast
  - reciprocal_approx_fast (custom DVE op) produces garbage on HW
  - matmul free dim capped at 512; 2-bank psum tiles need bank-aligned halves
"""

from contextlib import ExitStack
from dataclasses import dataclass

import numpy as np
import ml_dtypes

import concourse.bass as bass  # noqa: F401
import concourse.tile as tile
from concourse import bacc, mybir


@dataclass
class Cfg:
    D: int = 1024      # model dim
    S: int = 2048      # sequence length (queries == keys)
    HL: int = 8        # heads per core
    DK: int = 64       # head dim
    S_BLK: int = 512   # query block (matmul free dim)
    T_BLK: int = 512   # t block in projection phase

    @property
    def DC(self):
        return self.D // 128

    @property
    def NSB(self):
        return self.S // self.S_BLK

    @property
    def TBn(self):
        return self.S // self.T_BLK

    @property
    def TCn(self):
        return self.S // 128

    @property
    def JW(self):
        return self.HL * self.DK

    @property
    def JC(self):
        return self.JW // 128

    @property
    def VW(self):
        return self.DK + 1

    @property
    def OB(self):
        return min(512, self.D)


DT_NP = {
    mybir.dt.bfloat16: ml_dtypes.bfloat16,
    mybir.dt.float32: np.float32,
    mybir.dt.float32r: np.float32,
}


def build_nc(cfg: Cfg, DT=mybir.dt.bfloat16, num_devices: int = 8):
    c = cfg
    f32 = mybir.dt.float32
    EXPDT = DT if DT == mybir.dt.bfloat16 else f32
    SCALE = 1.0 / float(np.sqrt(c.DK))
    nc = bacc.Bacc("TRN2", target_bir_lowering=False, debug=False,
                   num_devices=num_devices)

    xqT = nc.dram_tensor("xqT", [c.D, c.S], DT, kind="ExternalInput").ap()
    xkT = nc.dram_tensor("xkT", [c.D, c.S], DT, kind="ExternalInput").ap()
    xvT = nc.dram_tensor("xvT", [c.D, c.S], DT, kind="ExternalInput").ap()
    wq_d = nc.dram_tensor("wq", [c.D, c.JW], DT, kind="ExternalInput").ap()
    wk_d = nc.dram_tensor("wk", [c.D, c.JW], DT, kind="ExternalInput").ap()
    wv_d = nc.dram_tensor("wv", [c.D, c.JW], DT, kind="ExternalInput").ap()
    wo_d = nc.dram_tensor("woutT", [c.JW, c.D], DT, kind="ExternalInput").ap()
    out_d = nc.dram_tensor("out", [c.S, c.D], f32, kind="ExternalOutput").ap()

    with tile.TileContext(nc) as tc, ExitStack() as es:
        wpool = es.enter_context(tc.tile_pool(name="weights", bufs=1))
        kvpool = es.enter_context(tc.tile_pool(name="kv", bufs=1))
        xpool = es.enter_context(tc.tile_pool(name="x", bufs=2))
        qpool = es.enter_context(tc.tile_pool(name="q", bufs=2))
        epool = es.enter_context(tc.tile_pool(name="exp", bufs=2))
        cpool = es.enter_context(tc.tile_pool(name="cat", bufs=2))
        opool = es.enter_context(tc.tile_pool(name="o", bufs=2))
        rpool = es.enter_context(tc.tile_pool(name="r", bufs=1))
        pspool = es.enter_context(tc.tile_pool(name="ps", bufs=2, space="PSUM"))
        pvpool = es.enter_context(tc.tile_pool(name="pv", bufs=2, space="PSUM"))
        fppool = es.enter_context(tc.tile_pool(name="fp", bufs=2, space="PSUM"))
        stpool = es.enter_context(tc.tile_pool(name="st", bufs=2))

        def load_w_dmaj(dram, width, tag):
            t = wpool.tile([128, c.DC * width], DT, tag=tag, name=tag)
            for d in range(c.DC):
                eng = nc.sync if d % 2 == 0 else nc.gpsimd
                eng.dma_start(t[:, d * width:(d + 1) * width],
                              dram[d * 128:(d + 1) * 128, :])
            return t

        def load_x_blk(dram, blk, width, name):
            t = xpool.tile([128, c.DC * width], DT, tag="x", name=name)
            for d in range(c.DC):
                eng = nc.sync if d % 2 == 0 else nc.gpsimd
                eng.dma_start(
                    t[:, d * width:(d + 1) * width],
                    dram[d * 128:(d + 1) * 128, blk * width:(blk + 1) * width])
            return t

        # ---- kT (wk + first xk DMAs lead the queue) ----
        wk_sb = load_w_dmaj(wk_d, c.JW, "wk")
        kT_sb = kvpool.tile([128, c.JC * c.S], DT)
        NT = c.T_BLK
        for tb in range(c.TBn):
            xk = load_x_blk(xkT, tb, NT, f"xk{tb}")
            for jc in range(c.JC):
                ps = pspool.tile([128, NT], f32, tag="ps", name=f"psk{tb}_{jc}")
                for d in range(c.DC):
                    nc.tensor.matmul(
                        ps[:],
                        wk_sb[:, d * c.JW + jc * 128: d * c.JW + (jc + 1) * 128],
                        xk[:, d * NT:(d + 1) * NT],
                        start=(d == 0), stop=(d == c.DC - 1))
                nc.vector.tensor_copy(
                    kT_sb[:, jc * c.S + tb * NT: jc * c.S + (tb + 1) * NT], ps[:])

        wq_sb = load_w_dmaj(wq_d, c.JW, "wq")

        # v' is emitted later (inside the first pipeline unit); declare here.
        v_sb = kvpool.tile([128, c.TCn * c.HL * c.VW], DT)
        wo_sb_box = {}

        def emit_v_phase():
            wv_sb = load_w_dmaj(wv_d, c.JW, "wv")
            nc.gpsimd.memset(v_sb[:], 1.0)  # ones columns preset
            for tb in range(c.TBn):
                xv = load_x_blk(xvT, tb, NT, f"xv{tb}")
                for tt in range(NT // 128):
                    g = tb * (NT // 128) + tt
                    ps = pspool.tile([128, c.JW], f32, tag="ps", name=f"psv{g}")
                    for d in range(c.DC):
                        nc.tensor.matmul(
                            ps[:],
                            xv[:, d * NT + tt * 128: d * NT + (tt + 1) * 128],
                            wv_sb[:, d * c.JW:(d + 1) * c.JW],
                            start=(d == 0), stop=(d == c.DC - 1))
                    dst = v_sb[:, g * c.HL * c.VW:(g + 1) * c.HL * c.VW]
                    dst3 = dst.rearrange("p (h w) -> p h w", w=c.VW)[:, :, 0:c.DK]
                    src3 = ps[:].rearrange("p (h w) -> p h w", w=c.DK)
                    nc.vector.tensor_copy(dst3, src3)
            # wo load rides behind the v-phase traffic, ahead of first out-proj
            wo_sb = wpool.tile([128, c.JC * c.D], DT, tag="wo", name="wo")
            for ic in range(c.JC):
                nc.sync.dma_start(wo_sb[:, ic * c.D:(ic + 1) * c.D],
                                  wo_d[ic * 128:(ic + 1) * 128, :])
            wo_sb_box["wo"] = wo_sb

        def emit_qT_mms(sb, xq, qT):
            """Filler closures: 32 MMs; the last per jc copies psum -> qT
            chunk jc (head A rows 0:64, head B rows 64:128 — natural layout,
            consumed by the row-tiled scores matmuls)."""
            ops = []
            psq_box = {}

            def mk(jc, d):
                def op():
                    if d == 0:
                        psq_box[jc] = fppool.tile([128, c.S_BLK], f32, tag="fp",
                                                  name=f"psq{sb}_{jc}")
                    nc.tensor.matmul(
                        psq_box[jc][:],
                        wq_sb[:, d * c.JW + jc * 128: d * c.JW + (jc + 1) * 128],
                        xq[:, d * c.S_BLK:(d + 1) * c.S_BLK],
                        start=(d == 0), stop=(d == c.DC - 1))
                    if d == c.DC - 1:
                        nc.vector.tensor_copy(
                            qT[:, jc * c.S_BLK:(jc + 1) * c.S_BLK],
                            psq_box[jc][:])
                return op
            for jc in range(c.JC):
                for d in range(c.DC):
                    ops.append(mk(jc, d))
            return ops

        def emit_outproj_mms(sb, catT):
            """Filler closures: per (sc, oc): 4 ic-MMs into a 1-bank psum,
            then copy + DMA."""
            ops = []
            po_box = {}

            def mk(sc, oc, ic):
                def op():
                    if ic == 0:
                        po_box[(sc, oc)] = fppool.tile(
                            [128, c.OB], f32, tag="fp", name=f"po{sb}_{sc}_{oc}")
                    po = po_box[(sc, oc)]
                    nc.tensor.matmul(
                        po[:],
                        catT[:, ic * c.S_BLK + sc * 128:
                             ic * c.S_BLK + (sc + 1) * 128],
                        wo_sb_box["wo"][:, ic * c.D + oc * c.OB:
                                        ic * c.D + (oc + 1) * c.OB],
                        start=(ic == 0), stop=(ic == c.JC - 1))
                    if ic == c.JC - 1:
                        ot = opool.tile([128, c.OB], f32, tag="ot",
                                        name=f"ot{sb}_{sc}_{oc}")
                        nc.vector.tensor_copy(ot[:], po[:])
                        nc.sync.dma_start(
                            out_d[sb * c.S_BLK + sc * 128:
                                  sb * c.S_BLK + (sc + 1) * 128,
                                  oc * c.OB:(oc + 1) * c.OB],
                            ot[:])
                return op
            for sc in range(c.S_BLK // 128):
                for oc in range(c.D // c.OB):
                    for ic in range(c.JC):
                        ops.append(mk(sc, oc, ic))
            return ops

        def emit_pv_chunk(u, t0, nt):
            W2 = 2 * c.S_BLK
            for t in range(t0, t0 + nt):
                nc.tensor.matmul(
                    u["pvA"][0:c.VW, :],
                    v_sb[:, t * c.HL * c.VW + (2 * u["hp"]) * c.VW:
                         t * c.HL * c.VW + (2 * u["hp"] + 1) * c.VW],
                    u["expAB"][:, t * W2:t * W2 + c.S_BLK],
                    start=(t == 0), stop=(t == c.TCn - 1))
            for t in range(t0, t0 + nt):
                nc.tensor.matmul(
                    u["pvB"][0:c.VW, :],
                    v_sb[:, t * c.HL * c.VW + (2 * u["hp"] + 1) * c.VW:
                         t * c.HL * c.VW + (2 * u["hp"] + 2) * c.VW],
                    u["expAB"][:, t * W2 + c.S_BLK:(t + 1) * W2],
                    start=(t == 0), stop=(t == c.TCn - 1))

        def emit_stage(u):
            # copy PV psum -> SBUF staging right away so the psum banks free
            # up for the next unit's PV (normalize then runs off-critical-path)
            sb, hp = u["sb"], u["hp"]
            u["stA"] = stpool.tile([c.VW, c.S_BLK], f32, tag="stA",
                                   name=f"stA{sb}_{hp}")
            u["stB"] = stpool.tile([c.VW, c.S_BLK], f32, tag="stB",
                                   name=f"stB{sb}_{hp}")
            nc.vector.tensor_copy(u["stA"][:], u["pvA"][0:c.VW, :])
            nc.vector.tensor_copy(u["stB"][:], u["pvB"][0:c.VW, :])

        def emit_normalize(u):
            sb, hp = u["sb"], u["hp"]
            stA, stB, catT = u["stA"], u["stB"], u["catT"]
            rtiA = rpool.tile([1, c.S_BLK], f32, tag="rtiA", name=f"rtiA{sb}_{hp}")
            rtiB = rpool.tile([1, c.S_BLK], f32, tag="rtiB", name=f"rtiB{sb}_{hp}")
            # NB: cross-partition (row 64 -> row 0) — verified OK on HW for
            # InstReciprocal specifically.
            nc.vector.reciprocal(rtiA[:], stA[c.DK:c.DK + 1, :])
            nc.vector.reciprocal(rtiB[:], stB[c.DK:c.DK + 1, :])
            rbA = rpool.tile([c.DK, c.S_BLK], f32, tag="rbA", name=f"rbA{sb}_{hp}")
            rbB = rpool.tile([c.DK, c.S_BLK], f32, tag="rbB", name=f"rbB{sb}_{hp}")
            nc.gpsimd.partition_broadcast(rbA[:], rtiA[:])
            nc.gpsimd.partition_broadcast(rbB[:], rtiB[:])
            nc.vector.tensor_mul(
                catT[0:c.DK, hp * c.S_BLK:(hp + 1) * c.S_BLK],
                stA[0:c.DK, :], rbA[:])
            nc.vector.tensor_mul(
                catT[64:64 + c.DK, hp * c.S_BLK:(hp + 1) * c.S_BLK],
                stB[0:c.DK, :], rbB[:])

        # ---- the main (sb, hp) software pipeline with per-th fillers ----
        units = [(sb, hp) for sb in range(c.NSB) for hp in range(c.JC)]
        # fillers[idx] = list of MM closures to interleave into unit idx's
        # scores loop (2 per th).
        fillers = [[] for _ in units]
        prev = None
        qT_tiles = {}
        cat_tiles = {}
        xq_tiles = {0: load_x_blk(xqT, 0, c.S_BLK, "xq0")}

        # sb=0 prologue: qT(0) emitted inline (dedicated MMs)
        qT_tiles[0] = qpool.tile([128, c.JC * c.S_BLK], DT, tag="qT",
                                 name="qT0")
        for op in emit_qT_mms(0, xq_tiles[0], qT_tiles[0]):
            op()

        for idx, (sb, hp) in enumerate(units):
            if hp == 0:
                cat_tiles[sb] = cpool.tile([128, c.JC * c.S_BLK], DT, tag="cat",
                                           name=f"catT{sb}")
            # xq prefetch two units before the qT fillers consume it
            pf = None
            if sb == 0 and hp == max(0, c.JC - 3):
                pf = 1
            elif sb == 0 and hp == c.JC - 1:
                pf = 2
            elif sb >= 1 and hp == min(2, c.JC - 1):
                pf = sb + 2
            if pf is not None and pf < c.NSB and pf not in xq_tiles:
                xq_tiles[pf] = load_x_blk(xqT, pf, c.S_BLK, f"xq{pf}")
            if sb == 0 and hp == max(0, c.JC - 2) and sb + 1 < c.NSB:
                # sb0: qT(1) fillers in the last two units (after the v phase)
                qT_tiles[1] = qpool.tile([128, c.JC * c.S_BLK], DT,
                                         tag="qT", name="qT1")
                qops = emit_qT_mms(1, xq_tiles[1], qT_tiles[1])
                fillers[idx] += qops[:16]
                fillers[min(idx + 1, len(units) - 1)] += qops[16:]
            if sb >= 1 and hp == 0 and sb + 1 < c.NSB:
                # steady state: qT(sb+1) fillers spread over all four units of
                # this sb (8 MMs each) so no single unit's PE load spikes
                qT_tiles[sb + 1] = qpool.tile([128, c.JC * c.S_BLK], DT,
                                              tag="qT", name=f"qT{sb + 1}")
                qops = emit_qT_mms(sb + 1, xq_tiles[sb + 1], qT_tiles[sb + 1])
                for j in range(4):
                    lo, hi = j * 8, (j + 1) * 8
                    fillers[min(idx + j, len(units) - 1)] += qops[lo:hi]
            catT = cat_tiles[sb]
            qT = qT_tiles[sb]
            cur = {
                "sb": sb, "hp": hp, "catT": catT,
                # per-chunk interleaved [expA(512) | expB(512)] blocks; one
                # activation per chunk covers both heads so a single sem frees
                # the A+B score matmuls of a later chunk together (keeps the
                # row-tiled pair adjacent in the schedule -> concurrent tiles)
                "expAB": epool.tile([128, c.TCn * 2 * c.S_BLK], EXPDT,
                                    tag="expAB", name=f"expAB{sb}_{hp}"),
            }
            if prev is not None:
                prev["pvA"] = pvpool.tile([128, c.S_BLK], f32, tag="pv",
                                          name=f"pvA{prev['sb']}_{prev['hp']}")
                prev["pvB"] = pvpool.tile([128, c.S_BLK], f32, tag="pv",
                                          name=f"pvB{prev['sb']}_{prev['hp']}")
            flist = fillers[idx]
            fpos = 0
            for th in range(c.TCn // 2):
                for u in range(2):
                    t = 2 * th + u
                    kcol = slice(hp * c.S + t * 128, hp * c.S + (t + 1) * 128)
                    qcol = slice(hp * c.S_BLK, (hp + 1) * c.S_BLK)
                    # One 2-bank psum tile [A(512) | B(512)] per t-chunk:
                    # K=64 row-tiled pair (head A on PE rows 0-63, tile (0,0);
                    # head B on rows 64-127, tile (64,0)) lands in different
                    # banks of the same tile, and ONE exp covers both heads.
                    ps2 = pspool.tile([128, 2 * c.S_BLK], f32, tag="ps",
                                      name=f"ps2_{sb}_{hp}_{t}")
                    nc.tensor.matmul(
                        ps2[:, 0:c.S_BLK],
                        kT_sb[0:64, kcol], qT[0:64, qcol],
                        start=True, stop=True)
                    nc.tensor.matmul(
                        ps2[:, c.S_BLK:2 * c.S_BLK],
                        kT_sb[64:128, kcol], qT[64:128, qcol],
                        start=True, stop=True)
                    nc.scalar.activation(
                        cur["expAB"][:, t * 2 * c.S_BLK:(t + 1) * 2 * c.S_BLK],
                        ps2[:], mybir.ActivationFunctionType.Exp, scale=SCALE)
                if prev is not None:
                    emit_pv_chunk(prev, 2 * th, 2)
                    if th == c.TCn // 2 - 1:
                        # stage immediately: frees the pv psum banks before
                        # the filler copies clog the DVE queue
                        emit_stage(prev)
                # interleave filler MMs evenly across the th loop
                want = (len(flist) * (th + 1)) // (c.TCn // 2)
                while fpos < want:
                    flist[fpos]()
                    fpos += 1
            if prev is not None:
                emit_normalize(prev)
                if prev["hp"] == c.JC - 1:
                    # out-projection of prev's sb becomes fillers of later
                    # units (catT complete only now), spread over three units
                    oops = emit_outproj_mms(prev["sb"], prev["catT"])
                    base = idx + 2 if sb + 1 < c.NSB else idx + 1
                    if base < len(units):
                        splits = [(0, 12), (12, 22), (22, 32)]
                        for j, (lo, hi) in enumerate(splits):
                            fillers[min(base + j, len(units) - 1)] += oops[lo:hi]
                    else:
                        for op in oops:
                            op()
            if idx == 0:
                emit_v_phase()
            prev = cur
        # drain the pipeline
        prev["pvA"] = pvpool.tile([128, c.S_BLK], f32, tag="pv", name="pvA_last")
        prev["pvB"] = pvpool.tile([128, c.S_BLK], f32, tag="pv", name="pvB_last")
        emit_pv_chunk(prev, 0, c.TCn)
        emit_stage(prev)
        emit_normalize(prev)
        for op in emit_outproj_mms(prev["sb"], prev["catT"]):
            op()

    nc.compile()
    return nc


def shard_inputs(inputs: dict, cfg: Cfg, DT=mybir.dt.bfloat16):
    """Full inputs -> list of 8 per-core in_maps (numpy)."""
    npdt = DT_NP[DT]
    q, k, v = inputs["queries"], inputs["keys"], inputs["values"]
    Wq, Wk, Wv = inputs["Wq"], inputs["Wk"], inputs["Wv"]
    Wout = inputs["Wout"]
    B = q.shape[0]
    maps = []
    WoutT = np.ascontiguousarray(Wout.T)  # [i, o]
    for core in range(2 * B):
        b, half = divmod(core, 2)
        hs = slice(half * cfg.HL, (half + 1) * cfg.HL)
        i0 = half * cfg.JW
        maps.append({
            "xqT": np.ascontiguousarray(q[b].T).astype(npdt),
            "xkT": np.ascontiguousarray(k[b].T).astype(npdt),
            "xvT": np.ascontiguousarray(v[b].T).astype(npdt),
            "wq": np.ascontiguousarray(
                Wq[hs].transpose(1, 0, 2).reshape(cfg.D, cfg.JW)).astype(npdt),
            "wk": np.ascontiguousarray(
                Wk[hs].transpose(1, 0, 2).reshape(cfg.D, cfg.JW)).astype(npdt),
            "wv": np.ascontiguousarray(
                Wv[hs].transpose(1, 0, 2).reshape(cfg.D, cfg.JW)).astype(npdt),
            "woutT": np.ascontiguousarray(WoutT[i0:i0 + cfg.JW]).astype(npdt),
        })
    return maps


def gather_outputs(results, inputs):
    bout = inputs["bout"]
    B = inputs["queries"].shape[0]
    outs = []
    for b in range(B):
        outs.append(results[2 * b]["out"] + results[2 * b + 1]["out"] + bout)
    return np.stack(outs).astype(np.float32)


def percore_reference(in_map: dict, cfg: Cfg):
    """Numpy reference of what one core should produce (fp32 math)."""
    c = cfg
    xq = in_map["xqT"].astype(np.float32).T   # [S, D]
    xk = in_map["xkT"].astype(np.float32).T
    xv = in_map["xvT"].astype(np.float32).T
    wq = in_map["wq"].astype(np.float32)      # [D, JW]
    wk = in_map["wk"].astype(np.float32)
    wv = in_map["wv"].astype(np.float32)
    wo = in_map["woutT"].astype(np.float32)   # [JW, D]
    q = xq @ wq                               # [S, JW]
    k = xk @ wk
    v = xv @ wv
    cat = np.zeros((c.S, c.JW), dtype=np.float32)
    for h in range(c.HL):
        sl = slice(h * c.DK, (h + 1) * c.DK)
        s = (q[:, sl] @ k[:, sl].T) / np.sqrt(c.DK)
        e = np.exp(s)
        p = e / e.sum(axis=1, keepdims=True)
        cat[:, sl] = p @ v[:, sl]
    return cat @ wo

# ----------------------------------------------------------------------------
# Self-contained entry point: kernel(**inputs) -> full [B, S, D] output.
# ----------------------------------------------------------------------------
_NC_CACHE = {}


def _get_nc():
    key = "attn"
    if key not in _NC_CACHE:
        _NC_CACHE[key] = build_nc(Cfg(), mybir.dt.bfloat16, num_devices=8)
    return _NC_CACHE[key]


def kernel(**inputs):
    """Full (unsharded) inputs -> full [4, 2048, 1024] float32 output.

    Shards across the 8 NeuronCores as (batch x head-half), runs the Bass
    kernel SPMD, and gathers: out[b] = partial(core 2b) + partial(core 2b+1)
    + bias (row-sharded fc_out -> partial-sum reduction at gather time).
    """
    from concourse.bass_utils import run_bass_kernel_spmd

    inputs = {k: np.asarray(v) for k, v in inputs.items()}
    cfg = Cfg()
    nc = _get_nc()
    maps = shard_inputs(inputs, cfg, mybir.dt.bfloat16)
    res = run_bass_kernel_spmd(nc, maps, core_ids=list(range(8)), trace=False)
    return gather_outputs(res.results, inputs)

